# revision 14
# baseline (speedup 1.0000x reference)
"""CapAttention Trainium2 kernel: 8-core data-parallel over batch.

Per core (4 batches): QKV projection -> masked/adj-biased attention
(transposed-score layout, softmax sums via ones-augmented V) -> capsule
routing (3 iters) with probs-folded flat matmuls; softmax normalization
folded into routing via reciprocal sums.
"""
import numpy as np
from contextlib import ExitStack

import concourse.bass as bass
import concourse.bacc as bacc
import concourse.tile as tile
from concourse import mybir
from concourse.bass_utils import run_bass_kernel_spmd
from concourse.masks import make_identity

F32 = mybir.dt.float32
F32R = mybir.dt.float32r
BF16 = mybir.dt.bfloat16
I32 = mybir.dt.int32
AF = mybir.ActivationFunctionType
ALU = mybir.AluOpType
AXX = mybir.AxisListType.X

B, S, D, H, DK = 32, 512, 640, 10, 64
NCORES = 8
BL = B // NCORES          # 4 local batches
P = 128
NCH = D // P              # 5 chunks of 128 along D
SC = S // P               # 4 chunks of 128 along S


def _t(nc, psum_slice, in_slice, ident):
    nc.tensor.transpose(psum_slice, in_slice, ident)


def build(layer_val: float, has_bias: bool):
    adj_c = 1.0 / (layer_val + 1.0)
    nc = bacc.Bacc(None, target_bir_lowering=False)

    dq = nc.dram_tensor("query", [BL, S, D], F32, kind="ExternalInput")
    dk = nc.dram_tensor("key", [BL, S, D], F32, kind="ExternalInput")
    dv = nc.dram_tensor("value", [BL, S, D], F32, kind="ExternalInput")
    dmask = nc.dram_tensor("mask", [BL, 1, S, S], I32, kind="ExternalInput")
    dadj = nc.dram_tensor("adj", [BL, S, S], F32, kind="ExternalInput")
    dwq = nc.dram_tensor("Wq", [D, D], F32, kind="ExternalInput")
    dwk = nc.dram_tensor("Wk", [D, D], F32, kind="ExternalInput")
    dwv = nc.dram_tensor("Wv", [D, D], F32, kind="ExternalInput")
    dbq = nc.dram_tensor("bq", [D], F32, kind="ExternalInput")
    dbk = nc.dram_tensor("bk", [D], F32, kind="ExternalInput")
    dbv = nc.dram_tensor("bv", [D], F32, kind="ExternalInput")
    drw = nc.dram_tensor("route_weights", [1, H, DK, D], F32, kind="ExternalInput")
    dout = nc.dram_tensor("out", [BL, S, D], F32, kind="ExternalOutput")

    with tile.TileContext(nc) as tc, ExitStack() as ctx:
        const = ctx.enter_context(tc.tile_pool(name="const", bufs=1))
        psum1 = ctx.enter_context(tc.tile_pool(name="psum1", bufs=4, space="PSUM"))
        psum2 = ctx.enter_context(tc.tile_pool(name="psum2", bufs=2, space="PSUM"))

        ident = const.tile([P, P], F32)
        make_identity(nc, ident)
        ident_r = const.tile([P, P], F32R)
        nc.vector.tensor_copy(ident_r, ident)
        ident_b = const.tile([P, P], BF16)
        nc.vector.tensor_copy(ident_b, ident)

        eps_t = const.tile([P, 1], F32)
        nc.vector.memset(eps_t, 1e-9)

        # per-partition bias vectors for ACT-fused projection bias
        bqT = const.tile([P, NCH], F32)
        bkT = const.tile([P, NCH], F32)
        nc.sync.dma_start(out=bqT, in_=dbq.ap().rearrange("(c p) -> p c", p=P))
        nc.sync.dma_start(out=bkT, in_=dbk.ap().rearrange("(c p) -> p c", p=P))
        bqTs = const.tile([P, NCH], F32)
        nc.vector.tensor_scalar_mul(bqTs, bqT, 0.125)
        bv_r = const.tile([1, D], F32R)
        ones_f = const.tile([1, P], F32)
        nc.vector.memset(ones_f, 1.0)
        ones_col = const.tile([1, P], F32R)
        nc.vector.tensor_copy(ones_col, ones_f)

        WqT = const.tile([P, NCH, D], F32R)
        WkT = const.tile([P, NCH, D], F32R)
        WvT = const.tile([P, NCH, D], F32R)
        wflat = const.tile([P, NCH, D], F32R)
        wflatT = const.tile([P, NCH, D], F32R)

        with tc.tile_pool(name="wstage", bufs=1) as wst:
            bvst = wst.tile([1, D], F32, tag="bvst")
            nc.sync.dma_start(out=bvst, in_=dbv.ap().rearrange("(one d) -> one d", one=1))
            nc.vector.tensor_copy(bv_r, bvst)
            for wsrc, wdstT in ((dwq, WqT), (dwk, WkT), (dwv, WvT)):
                wraw = wst.tile([P, NCH, D], F32, tag="wraw")
                nc.sync.dma_start(
                    out=wraw, in_=wsrc.ap().rearrange("(j p) d -> p j d", p=P))
                # WT[:, c, 128j:128j+128] = W[128j.., 128c..]^T
                for c in range(NCH):
                    pt = psum2.tile([P, D], F32, tag="b2")
                    for j in range(NCH):
                        _t(nc, pt[:, j * P:(j + 1) * P],
                           wraw[:, j, c * P:(c + 1) * P], ident)
                    nc.vector.tensor_copy(wdstT[:, c, :], pt)
            # route weights, flattened [(r i), o]
            wfst = wst.tile([P, NCH, D], F32, tag="wraw")
            nc.sync.dma_start(
                out=wfst,
                in_=drw.ap().rearrange("one r i o -> (one r i) o")
                            .rearrange("(c p) o -> p c o", p=P))
            nc.vector.tensor_copy(wflat, wfst)
            for oc in range(NCH):
                pt = psum2.tile([P, D], F32, tag="b2")
                for c in range(NCH):
                    _t(nc, pt[:, c * P:(c + 1) * P],
                       wfst[:, c, oc * P:(oc + 1) * P], ident)
                nc.vector.tensor_copy(wflatT[:, oc, :], pt)

        xraw_p = ctx.enter_context(tc.tile_pool(name="xraw", bufs=1))
        xt_p = ctx.enter_context(tc.tile_pool(name="xt", bufs=1))
        qkt_p = ctx.enter_context(tc.tile_pool(name="qkt", bufs=1))
        vaug_p = ctx.enter_context(tc.tile_pool(name="vaug", bufs=1))
        am_p = ctx.enter_context(tc.tile_pool(name="am", bufs=2))
        bias_p = ctx.enter_context(tc.tile_pool(name="bias", bufs=1))
        expt_p = ctx.enter_context(tc.tile_pool(name="expt", bufs=1))
        sco_p = ctx.enter_context(tc.tile_pool(name="sco", bufs=2))
        xtb_p = ctx.enter_context(tc.tile_pool(name="xtb", bufs=1))
        sum_p = ctx.enter_context(tc.tile_pool(name="sump", bufs=1))
        st1_p = ctx.enter_context(tc.tile_pool(name="st1", bufs=2))
        dram_p = ctx.enter_context(tc.tile_pool(name="dramst", bufs=2, space="DRAM"))
        rt_p = ctx.enter_context(tc.tile_pool(name="rt", bufs=2))
        z_p = ctx.enter_context(tc.tile_pool(name="z", bufs=1))
        scr_p = ctx.enter_context(tc.tile_pool(name="scr", bufs=1))
        small_p = ctx.enter_context(tc.tile_pool(name="small", bufs=16))
        outsb_p = ctx.enter_context(tc.tile_pool(name="outsb", bufs=1))

        for b in range(BL):
            # ---- load + transpose inputs ----
            xts = {}
            for name, src in (("q", dq), ("k", dk), ("v", dv)):
                raw = xraw_p.tile([P, SC, D], F32, tag="xraw")
                nc.sync.dma_start(
                    out=raw, in_=src.ap()[b].rearrange("(m p) d -> p m d", p=P))
                xt = xt_p.tile([P, NCH, S], F32R, tag=f"xt{name}")
                for c in range(NCH):
                    pt = psum1.tile([P, S], F32, tag="b1")
                    for m in range(SC):
                        _t(nc, pt[:, m * P:(m + 1) * P],
                           raw[:, m, c * P:(c + 1) * P], ident)
                    nc.vector.tensor_copy(xt[:, c, :], pt)
                xts[name] = xt

            # ---- projections ----
            qTs = qkt_p.tile([P, NCH, S], F32R, tag="qTs")
            kTs = qkt_p.tile([P, NCH, S], F32R, tag="kTs")
            for xt, wT, dst, bias_ap, scale in (
                    (xts["q"], WqT, qTs, bqTs, 0.125),
                    (xts["k"], WkT, kTs, bkT, 1.0)):
                for j in range(NCH):
                    pp = psum1.tile([P, S], F32, tag="b1")
                    for c in range(NCH):
                        nc.tensor.matmul(pp, wT[:, c, j * P:(j + 1) * P],
                                         xt[:, c, :],
                                         start=(c == 0), stop=(c == NCH - 1))
                    if has_bias:
                        nc.scalar.activation(out=dst[:, j, :], in_=pp, func=AF.Copy,
                                             bias=bias_ap[:, j:j + 1], scale=scale)
                    else:
                        nc.scalar.activation(out=dst[:, j, :], in_=pp, func=AF.Copy,
                                             scale=scale)
            vaug = vaug_p.tile([P, SC, H, DK + 1], BF16, tag="vaug")
            nc.gpsimd.memset(vaug[:, :, :, DK:DK + 1], 1.0)
            for m in range(SC):
                pv = psum2.tile([P, D], F32, tag="b2")
                for lo, hi in ((0, 512), (512, 640)):
                    for c in range(NCH):
                        nc.tensor.matmul(pv[:, lo:hi],
                                         xts["v"][:, c, m * P:(m + 1) * P],
                                         WvT[:, c, lo:hi],
                                         start=(c == 0),
                                         stop=(c == NCH - 1 and not has_bias))
                if has_bias:
                    for lo, hi in ((0, 512), (512, 640)):
                        nc.tensor.matmul(pv[:, lo:hi],
                                         ones_col, bv_r[:, lo:hi],
                                         start=False, stop=(hi == 640))
                nc.scalar.activation(out=vaug[:, m, :, 0:DK],
                                     in_=pv.rearrange("p (h i) -> p h i", h=H),
                                     func=AF.Copy)

            # ---- additive bias tile: adj/(layer+1) + (mask-1)*1e9, then transpose ----
            adjr = am_p.tile([P, SC, S], F32, tag="am")
            nc.sync.dma_start(
                out=adjr, in_=dadj.ap()[b].rearrange("(m p) t -> p m t", p=P))
            maskr = am_p.tile([P, SC, S], I32, tag="am")
            nc.sync.dma_start(
                out=maskr, in_=dmask.ap()[b, 0].rearrange("(m p) t -> p m t", p=P))
            biasf = bias_p.tile([P, SC, S], BF16, tag="biasf")
            nc.gpsimd.tensor_copy(biasf, maskr)
            nc.gpsimd.tensor_scalar(out=biasf, in0=biasf, scalar1=1.0, scalar2=1e9,
                                    op0=ALU.subtract, op1=ALU.mult)
            nc.vector.scalar_tensor_tensor(out=biasf, in0=adjr, scalar=adj_c,
                                           in1=biasf, op0=ALU.mult, op1=ALU.add)
            biasT = bias_p.tile([P, SC, S], BF16, tag="biasT")
            for tch in range(SC):
                pbt = psum1.tile([P, S], BF16, tag="b1")
                for m in range(SC):
                    _t(nc, pbt[:, m * P:(m + 1) * P],
                       biasf[:, m, tch * P:(tch + 1) * P], ident_b)
                nc.vector.tensor_copy(biasT[:, tch, :], pbt)

            # ---- attention per head ----
            xTb = xtb_p.tile([P, NCH, S], F32R, tag="xTb")
            sums_dram = dram_p.tile([H, S], F32, tag="sumsd")
            for h in range(H):
                jh, oh = divmod(DK * h, P)
                expT = expt_p.tile([P, SC, S], BF16, tag="expT")
                for tch in range(SC):
                    psc = psum1.tile([P, S], F32, tag="b1")
                    nc.tensor.matmul(psc,
                                     kTs[oh:oh + DK, jh, tch * P:(tch + 1) * P],
                                     qTs[oh:oh + DK, jh, :],
                                     start=True, stop=True)
                    scob = sco_p.tile([P, S], F32, tag="scob")
                    nc.vector.tensor_add(scob, psc, biasT[:, tch, :])
                    nc.scalar.activation(out=expT[:, tch, :], in_=scob, func=AF.Exp)
                px = psum1.tile([DK + 1, S], F32, tag="b1")
                for tch in range(SC):
                    nc.tensor.matmul(px, vaug[:, tch, h, :], expT[:, tch, :],
                                     start=(tch == 0), stop=(tch == SC - 1))
                nc.vector.tensor_copy(
                    xTb[(h % 2) * DK:(h % 2) * DK + DK, h // 2, :], px[0:DK, :])
                srow = st1_p.tile([1, S], F32, tag="srow")
                nc.vector.tensor_copy(srow, px[DK:DK + 1, :])
                nc.sync.dma_start(out=sums_dram[h:h + 1, :], in_=srow)

            # ---- routing ----
            sumsT = sum_p.tile([P, SC, H], F32, tag="sumsT")
            for tch in range(SC):
                nc.sync.dma_start(
                    out=sumsT[:, tch, :],
                    in_=sums_dram[:, tch * P:(tch + 1) * P].rearrange("h p -> p h"))
            rs = sum_p.tile([P, SC, H], F32, tag="rs")
            nc.vector.reciprocal(rs.rearrange("p a b -> p (a b)"),
                                 sumsT.rearrange("p a b -> p (a b)"))

            for tch in range(SC):
                rst = rs[:, tch, :]
                z = z_p.tile([P, D], F32R, tag="z")
                pz = psum2.tile([P, D], F32R, tag="b2")
                for c in range(NCH):
                    _t(nc, pz[:, c * P:(c + 1) * P],
                       xTb[:, c, tch * P:(tch + 1) * P], ident_r)
                nc.vector.tensor_copy(z, pz)
                z3 = z.rearrange("p (r i) -> p r i", r=H)

                logits = small_p.tile([P, H], F32, tag="logits")
                s_sb = z_p.tile([P, D], F32R, tag="s_sb")
                alpha = None
                for it in range(3):
                    w = small_p.tile([P, H], F32, tag="w")
                    if it == 0:
                        nc.vector.tensor_scalar_mul(w, rst, 1.0 / H)
                    else:
                        el = small_p.tile([P, H], F32, tag="el")
                        nc.scalar.activation(out=el, in_=logits, func=AF.Exp)
                        se = small_p.tile([P, 1], F32, tag="se")
                        nc.vector.reduce_sum(out=se, in_=el, axis=AXX)
                        rse = small_p.tile([P, 1], F32, tag="rse")
                        nc.vector.reciprocal(rse, se)
                        probs = small_p.tile([P, H], F32, tag="probs")
                        nc.vector.tensor_scalar_mul(probs, el, rse)
                        nc.vector.tensor_mul(w, probs, rst)
                    zs = scr_p.tile([P, D], F32R, tag="zs")
                    nc.vector.tensor_tensor(
                        out=zs.rearrange("p (r i) -> p r i", r=H), in0=z3,
                        in1=w.broadcast_to([P, H, DK]), op=ALU.mult)
                    pzs = psum2.tile([P, D], F32R, tag="b2")
                    for c in range(NCH):
                        _t(nc, pzs[:, c * P:(c + 1) * P],
                           zs[:, c * P:(c + 1) * P], ident_r)
                    zsT = rt_p.tile([P, D], F32R, tag="zsT")
                    nc.vector.tensor_copy(zsT, pzs)
                    ps_ = psum2.tile([P, D], F32, tag="b2")
                    for lo, hi in ((0, 512), (512, 640)):
                        for c in range(NCH):
                            nc.tensor.matmul(
                                ps_[:, lo:hi],
                                zsT[:, c * P:(c + 1) * P],
                                wflat[:, c, lo:hi],
                                start=(c == 0), stop=(c == NCH - 1))
                    nc.vector.tensor_copy(s_sb, ps_)
                    sn = small_p.tile([P, 1], F32, tag="sn")
                    sq = scr_p.tile([P, D], F32, tag="sq")
                    nc.scalar.activation(out=sq, in_=ps_, func=AF.Square,
                                         accum_out=sn)
                    rt_ = small_p.tile([P, 1], F32, tag="rt_")
                    nc.scalar.activation(out=rt_, in_=sn, func=AF.Sqrt,
                                         bias=eps_t[:, 0:1])
                    d_ = small_p.tile([P, 1], F32, tag="d_")
                    nc.vector.tensor_scalar_add(d_, sn, 1.0)
                    dn = small_p.tile([P, 1], F32, tag="dn")
                    nc.vector.tensor_mul(dn, d_, rt_)
                    rec = small_p.tile([P, 1], F32, tag="rec")
                    nc.vector.reciprocal(rec, dn)
                    alpha = small_p.tile([P, 1], F32, tag="alpha")
                    nc.vector.tensor_mul(alpha, sn, rec)

                    if it < 2:
                        # out_i = alpha * s ; a = (z . (Wflat^T out_i)) * rs
                        nc.vector.tensor_scalar_mul(s_sb, s_sb, alpha)
                        pot = psum2.tile([P, D], F32R, tag="b2")
                        for c in range(NCH):
                            _t(nc, pot[:, c * P:(c + 1) * P],
                               s_sb[:, c * P:(c + 1) * P], ident_r)
                        outT = rt_p.tile([P, D], F32R, tag="zsT")
                        nc.vector.tensor_copy(outT, pot)
                        ph = psum2.tile([P, D], F32, tag="b2")
                        for lo, hi in ((0, 512), (512, 640)):
                            for c in range(NCH):
                                nc.tensor.matmul(
                                    ph[:, lo:hi],
                                    outT[:, c * P:(c + 1) * P],
                                    wflatT[:, c, lo:hi],
                                    start=(c == 0), stop=(c == NCH - 1))
                        tmp = scr_p.tile([P, D], F32, tag="sq")
                        nc.vector.tensor_mul(tmp, z, ph)
                        a_u = small_p.tile([P, H], F32, tag="a_u")
                        nc.vector.reduce_sum(
                            out=a_u, in_=tmp.rearrange("p (r i) -> p r i", r=H),
                            axis=AXX)
                        if it == 0:
                            nc.vector.tensor_mul(logits, a_u, rst)
                        else:
                            a_t = small_p.tile([P, H], F32, tag="a_t")
                            nc.vector.tensor_mul(a_t, a_u, rst)
                            nc.vector.tensor_add(logits, logits, a_t)
                    else:
                        outf = outsb_p.tile([P, D], F32, tag="outf")
                        nc.vector.tensor_scalar_mul(outf, s_sb, alpha)
                        nc.sync.dma_start(
                            out=dout.ap()[b, tch * P:(tch + 1) * P, :], in_=outf)
    nc.finalize()
    return nc


_cache = {}


def _get_nc(layer_val: float, has_bias: bool):
    key = (layer_val, has_bias)
    if key not in _cache:
        _cache[key] = build(layer_val, has_bias)
    return _cache[key]


def kernel(**inputs) -> np.ndarray:
    ins = {k: np.asarray(v) for k, v in inputs.items()}
    layer_val = float(ins.pop("layer"))
    has_bias = bool(np.any(ins["bq"]) or np.any(ins["bk"]) or np.any(ins["bv"]))
    nc = _get_nc(layer_val, has_bias)

    shard_keys = ("query", "key", "value", "mask", "adj")
    in_maps = []
    for i in range(NCORES):
        m = {}
        for k, v in ins.items():
            if k in shard_keys:
                m[k] = np.ascontiguousarray(v[i * BL:(i + 1) * BL]).astype(
                    v.dtype, copy=False)
            else:
                m[k] = np.ascontiguousarray(v)
        in_maps.append(m)

    res = run_bass_kernel_spmd(nc, in_maps, core_ids=list(range(NCORES)))
    out = np.concatenate([res.results[i]["out"] for i in range(NCORES)], axis=0)
    return out
